# revision 29
# baseline (speedup 1.0000x reference)
"""Trainium2 Bass kernel for DecoderRNNWithAttention (teacher-forced LSTM decoder).

Key mathematical simplification: the attention block is an exact no-op.
The encoder output has a single spatial position, so softmax over that
axis is exactly 1.0 and context == features, independent of h. Hence:
  - the enc/dec/full attention projections never affect the output;
  - the input-side gate contributions Gx = X @ W_ih.T + (b_ih + b_hh)
    can be precomputed for all T steps in one batched matmul
    (X_t = [word_t ; features]);
  - the serial recurrence is only gates_t = Gx_t + h_t @ W_hh.T plus the
    LSTM elementwise cell; logits_t = h_{t+1} @ fcn_W.T + fcn_b.

Sharding: pure data-parallel over batch. 8 cores x 16 rows, no collectives.

Device layouts (all "transposed" so the partition dim is the feature dim):
  - gate dim 4H split into 32 slices of 128, permuted [i f o g] so one
    sigmoid covers cols 0:384 and one tanh covers cols 384:512 of the
    per-step [128, 512] gate tile (cols = slice-block * 16 batch).
  - recurrence MMs issue in [i f g o] group order and the LSTM cell is
    split fine-grained across vector/scalar so the elementwise chain
    hides under the matmul phase of the same step.
  - h state history Hall[128, t*128 + k*16 + b] (k = H-tile), written
    once per step as one [128, 128] tile; doubles as matmul rhs slices.
  - vocab projection: 50 weight chunks x 5 vocab tiles, logits written
    bf16 (bias added on host), one weight DMA + one out DMA per chunk.
"""

import numpy as np
import ml_dtypes

import concourse.bacc as bacc
import concourse.mybir as mybir
import concourse.tile as tile
from concourse.bass_utils import run_bass_kernel_spmd

B, T, E, H, V, ENC = 128, 25, 512, 1024, 32000, 400
NCORES = 8
BS = B // NCORES          # 16 batch rows per core
TB = T * BS               # 400 = matmul N for phase 1
ROWS = (T - 1) * BS       # 384 = matmul N for the vocab projection
KT = H // 128             # 8 K-tiles
GS = 4 * H // 128         # 32 gate slices
NCH = 50                  # fcn weight chunks (5 vocab tiles = 640 cols each)
CVT = 5                   # vocab tiles per chunk
VT = V // 128             # 250 vocab tiles (covered exactly by 50*5)
XDIM = E + ENC            # 912, padded to 1024

# torch LSTMCell gate order is [i f g o]; we want [i f o g] so sigmoid is
# one contiguous span. perm_src[j] = source slice for permuted block j.
PERM_SRC = list(range(0, 16)) + list(range(24, 32)) + list(range(16, 24))

CFG = {
    "p1": "bf16",    # phase-1 (Gx) matmul dtype
    "rec": "bf16",   # recurrence (W_hh) matmul dtype
    "fcn": "bf16",   # vocab projection matmul dtype
    "heat": 0,       # junk high-N matmuls per step (PE activity insurance)
}

_F32 = mybir.dt.float32
_DT = {"f32": mybir.dt.float32, "f32r": mybir.dt.float32r, "bf16": mybir.dt.bfloat16}
_NPDT = {"f32": np.float32, "f32r": np.float32, "bf16": ml_dtypes.bfloat16}

# recurrence MM group order: i+f and g first (feed the c-path), o last;
# each group accumulates into its own PSUM tile so its activation
# releases as soon as that group's matmuls land (per-tile dep
# granularity). Gx is folded into PSUM by one identity matmul per group
# (stationary = I_128, rhs = the contiguous Gx columns of step t), so
# the activations read PSUM directly and no vector adds exist at all.
# gxt layout is t-major: col = t*512 + j*16 + b.
P2_GROUPS = [
    ("fi", 0, 16),      # i,f slices: gxt cols 0:256, sigmoid
    ("gg", 24, 32),     # g slices:  gxt cols 384:512, tanh
    ("oo", 16, 24),     # o slices:  gxt cols 256:384, sigmoid (tail)
]


def build_nc(cfg=CFG):
    AF = mybir.ActivationFunctionType
    p1, rec, fcn = cfg["p1"], cfg["rec"], cfg["fcn"]
    NHEAT = cfg.get("heat", 1)
    IDENT = cfg.get("ident", True)

    nc = bacc.Bacc()
    xT_d = nc.dram_tensor("xT", [128, KT * TB], _DT[p1], kind="ExternalInput")
    wih_d = nc.dram_tensor("wih", [4, KT, 128, 1024], _DT[p1],
                           kind="ExternalInput")
    whh_d = nc.dram_tensor("whh", [128, KT * 4 * H], _DT[rec], kind="ExternalInput")
    fcnw_d = nc.dram_tensor("fcnw", [NCH, 128, KT * CVT * 128], _DT[fcn],
                            kind="ExternalInput")
    bsum_d = nc.dram_tensor("bsum", [128, GS], _F32, kind="ExternalInput")
    ident_d = nc.dram_tensor("ident", [128, 128], _DT[rec], kind="ExternalInput")
    out_d = nc.dram_tensor("out", [NCH, 128, CVT * ROWS], _DT[fcn],
                           kind="ExternalOutput")

    with tile.TileContext(nc) as tc:
        with (
            tc.tile_pool(name="pers", bufs=1) as pers,
            tc.tile_pool(name="psum", bufs=4, space="PSUM") as psum,
            tc.tile_pool(name="elem", bufs=2) as elem,
        ):
            hall = pers.tile([128, T * 128], _DT[rec])
            xt_sb = pers.tile([128, KT * TB], _DT[p1])
            bsum_sb = pers.tile([128, GS], _F32)
            ident_sb = pers.tile([128, 128], _DT[rec])
            # fcn weight chunks double-buffered in their own space so the
            # stream can start during the recurrence (no WAR on whh/gxt)
            fcnp = tc.alloc_tile_pool(name="fcnp", bufs=cfg.get("fcnb", 5))
            whhp = tc.alloc_tile_pool(name="whhp", bufs=1)
            gxtp = tc.alloc_tile_pool(name="gxtp", bufs=1)
            whh_sb = whhp.tile([128, KT * 4 * H], _DT[rec], name="whh_sb")
            # t-major Gx in the matmul dtype: col = t*512 + j*16 + b
            gxt = gxtp.tile([128, T * GS * BS], _DT[rec], name="gxt")
            gxt_t = gxt.rearrange("p (t j b) -> p t (j b)", t=T, j=GS, b=BS)

            nc.sync.dma_start(bsum_sb[:], bsum_d[:])
            nc.sync.dma_start(ident_sb[:], ident_d[:])
            nc.gpsimd.memset(hall[:], 0.0)

            # ---------------- Phase 1: Gx = X @ W_ih.T + (b_ih + b_hh) ----
            # k-sliced DMAs so the first matmul starts ~2us in instead of
            # waiting for the full 2MB quarter + X transfer
            with tc.tile_pool(name="wihp", bufs=2) as wihp:
                for quarter in range(4):
                    wih_sb = wihp.tile([128, KT * 1024], _DT[p1], tag="wih")
                    for k in range(KT):
                        if quarter == 0:
                            nc.sync.dma_start(xt_sb[:, k * TB:(k + 1) * TB],
                                              xT_d[:, k * TB:(k + 1) * TB])
                        nc.sync.dma_start(wih_sb[:, k * 1024:(k + 1) * 1024],
                                          wih_d[quarter, k])
                    for jj in range(8):
                        j = quarter * 8 + jj
                        ps = psum.tile([128, TB], _F32, tag="ps", name="ps", bufs=4)
                        for k in range(KT):
                            nc.tensor.matmul(
                                ps[:],
                                wih_sb[:, k * 1024 + jj * 128:
                                       k * 1024 + jj * 128 + 128],
                                xt_sb[:, k * TB:(k + 1) * TB],
                                start=(k == 0), stop=(k == KT - 1))
                        nc.scalar.activation(
                            gxt_t[:, :, j * BS:(j + 1) * BS],
                            ps.rearrange("p (t b) -> p t b", b=BS),
                            AF.Identity, bias=bsum_sb[:, j:j + 1])

            # W_hh load ordered after phase-1 inputs so phase 1 starts early
            for half in range(2):
                nc.sync.dma_start(whh_sb[:, half * 16384:(half + 1) * 16384],
                                  whh_d[:, half * 16384:(half + 1) * 16384])

            # ---------------- Phase 2: LSTM recurrence --------------------
            c_ab = [pers.tile([128, 128], _F32, name="c_a"),
                    pers.tile([128, 128], _F32, name="c_b")]

            # Gx-seed matmuls depend only on gxt (+ a WAR on the previous
            # step's activation reads), not on h — emitting step t+1's fi/gg
            # seeds right after step t's W-matmuls lets them fill the PE idle
            # during step t's elementwise tail.
            pstiles = {}

            def emit_ident(t, groups):
                for gname, j0, j1 in groups:
                    nsl = j1 - j0
                    ps_t = psum.tile([128, nsl * BS], _F32, tag=gname,
                                     name=gname, bufs=1)
                    nc.tensor.matmul(
                        ps_t[:], ident_sb[:],
                        gxt_t[:, t, j0 * BS:j1 * BS],
                        start=True, stop=False)
                    pstiles[(t, gname)] = ps_t

            c_prev = None
            for t in range(T):
                if t == 0:
                    gates_src = gxt_t[:, 0, :]  # [128, 512] contiguous
                    sig_if = elem.tile([128, 256], _F32, tag="sif", name="sif")
                    nc.scalar.activation(sig_if[:], gates_src[:, 0:256], AF.Sigmoid)
                    tg = elem.tile([128, 128], _F32, tag="tg", name="tg")
                    nc.scalar.activation(tg[:], gates_src[:, 384:512], AF.Tanh)
                    cn = c_ab[0]
                    nc.vector.tensor_mul(cn[:], sig_if[:, 0:128], tg[:])
                    thc = elem.tile([128, 128], _F32, tag="thc", name="thc")
                    nc.scalar.activation(thc[:], cn[:], AF.Tanh)
                    sig_o = elem.tile([128, 128], _F32, tag="so", name="so")
                    nc.scalar.activation(sig_o[:], gates_src[:, 256:384], AF.Sigmoid)
                    nc.vector.tensor_mul(hall[:, 0:128], sig_o[:], thc[:])
                    c_prev = cn
                    emit_ident(1, P2_GROUPS)
                else:
                    pst = {}
                    for gname, j0, j1 in P2_GROUPS:
                        ps_t = pstiles.pop((t, gname))
                        for ji, j in enumerate(range(j0, j1)):
                            for k in range(KT):
                                nc.tensor.matmul(
                                    ps_t[:, ji * BS:ji * BS + BS],
                                    whh_sb[:, k * 4096 + j * 128:
                                           k * 4096 + j * 128 + 128],
                                    hall[:, (t - 1) * 128 + k * BS:
                                         (t - 1) * 128 + k * BS + BS],
                                    start=False,
                                    stop=(ji == j1 - j0 - 1) and (k == KT - 1))
                        pst[gname] = ps_t
                        if gname == "oo" and t + 1 < T:
                            # next step's fi/gg seeds fill this step's tail
                            emit_ident(t + 1, P2_GROUPS[:2])
                    if t + 1 < T:
                        emit_ident(t + 1, P2_GROUPS[2:])
                    sig_if = elem.tile([128, 256], _F32, tag="sif", name="sif")
                    nc.scalar.activation(sig_if[:], pst["fi"][:], AF.Sigmoid)
                    tg = elem.tile([128, 128], _F32, tag="tg", name="tg")
                    nc.scalar.activation(tg[:], pst["gg"][:], AF.Tanh)
                    # c-path on vector while the o-group MMs still run
                    cn = c_ab[t % 2]
                    nc.vector.tensor_mul(cn[:], sig_if[:, 128:256], c_prev[:])
                    t1 = elem.tile([128, 128], _F32, tag="t1", name="t1")
                    nc.vector.tensor_mul(t1[:], sig_if[:, 0:128], tg[:])
                    nc.vector.tensor_add(cn[:], cn[:], t1[:])
                    thc = elem.tile([128, 128], _F32, tag="thc", name="thc")
                    nc.scalar.activation(thc[:], cn[:], AF.Tanh)
                    sig_o = elem.tile([128, 128], _F32, tag="so", name="so")
                    nc.scalar.activation(sig_o[:], pst["oo"][:], AF.Sigmoid)
                    nc.vector.tensor_mul(hall[:, t * 128:(t + 1) * 128],
                                         sig_o[:], thc[:])
                    # junk high-N matmuls keep some PE streaming activity in
                    # the elementwise tail (clock-gate insurance)
                    for _hi in range(NHEAT):
                        hps = psum.tile([128, 512], _F32, tag="heat",
                                        name="heat", bufs=1)
                        nc.tensor.matmul(hps[:], whh_sb[:, 0:128],
                                         whh_sb[:, 0:512],
                                         start=True, stop=True)
                    c_prev = cn

            # ---------------- Phase 3: logits = H @ fcn_W.T ---------------
            # fcnp lives OUTSIDE the whh/gxt space (allocated up front), so
            # its weight DMAs have no WAR deps on phase 2 and stream during
            # the recurrence. All weight DMAs are emitted before the matmul
            # loop; the pool ring paces chunk c+FCNB behind chunk c's reads.
            hall_r = hall.rearrange("p (t g) -> p t g", g=128)
            wts = []
            for c in range(NCH):
                wt = fcnp.tile([128, KT * CVT * 128], _DT[fcn], tag="fw",
                               name="fw")
                nc.sync.dma_start(wt[:], fcnw_d[c])
                wts.append(wt)
            hfp = tc.alloc_tile_pool(name="hfp", bufs=1)
            hf = []
            for k in range(KT):
                hfk = hfp.tile([128, T - 1, BS], _DT[fcn], name=f"hf{k}")
                nc.vector.tensor_copy(hfk[:], hall_r[:, 1:T, k * BS:(k + 1) * BS])
                hf.append(hfk.rearrange("p a b -> p (a b)"))
            with tc.tile_pool(name="outp", bufs=4) as outp:
                for c in range(NCH):
                    wt = wts[c]
                    ot = outp.tile([128, CVT * ROWS], _DT[fcn], tag="ot", name="ot")
                    for mi in range(CVT):
                        ps = psum.tile([128, ROWS], _F32, tag="ps", name="psf",
                                       bufs=4)
                        for k in range(KT):
                            nc.tensor.matmul(
                                ps[:],
                                wt[:, k * CVT * 128 + mi * 128:
                                   k * CVT * 128 + mi * 128 + 128],
                                hf[k],
                                start=(k == 0), stop=(k == KT - 1))
                        nc.vector.tensor_copy(ot[:, mi * ROWS:(mi + 1) * ROWS],
                                              ps[:])
                    nc.sync.dma_start(out_d[c], ot[:])
            hfp.release()
            gxtp.release()
            whhp.release()
            fcnp.release()

    nc.finalize()
    return nc


def _prep_shared(W_ih, W_hh, b_ih, b_hh, fcn_W, fcn_b, cfg):
    """Host-side layout transforms (no FLOPs beyond the bias sum)."""
    perm = np.concatenate([np.arange(s * 128, (s + 1) * 128) for s in PERM_SRC])
    p1np, recnp, fcnnp = _NPDT[cfg["p1"]], _NPDT[cfg["rec"]], _NPDT[cfg["fcn"]]

    wihT = np.zeros((H, 4 * H), np.float32)
    wihT[:XDIM, :] = np.asarray(W_ih, np.float32)[perm].T
    # [4 quarters, KT, 128, 1024]: per-(quarter, k) DMA granules
    wih_t = np.ascontiguousarray(
        wihT.reshape(KT, 128, 4, 1024).transpose(2, 0, 1, 3)
    ).astype(p1np)

    whhT = np.asarray(W_hh, np.float32)[perm].T  # [H, 4H]
    whh_t = np.ascontiguousarray(
        whhT.reshape(KT, 128, 4 * H).transpose(1, 0, 2).reshape(128, KT * 4 * H)
    ).astype(recnp)

    fw = np.asarray(fcn_W, np.float32)  # [V, H]
    fcnw_t = np.ascontiguousarray(
        fw.T.reshape(KT, 128, NCH, CVT * 128).transpose(2, 1, 0, 3)
        .reshape(NCH, 128, KT * CVT * 128)
    ).astype(fcnnp)

    bsum = (np.asarray(b_ih, np.float32) + np.asarray(b_hh, np.float32))[perm]
    bsum_t = np.ascontiguousarray(bsum.reshape(GS, 128).T)
    ident = np.eye(128, dtype=np.float32).astype(recnp)
    return {"wih": wih_t, "whh": whh_t, "fcnw": fcnw_t, "bsum": bsum_t,
            "ident": ident}


def _prep_core(features, captions, emb_W, core, cfg):
    p1np = _NPDT[cfg["p1"]]
    sl = slice(core * BS, (core + 1) * BS)
    feats = np.asarray(features, np.float32)[sl]          # [16, ENC]
    caps = np.asarray(captions)[sl]                       # [16, T]
    embW = np.asarray(emb_W, np.float32)

    words = np.empty((BS, T, E), np.float32)
    words[:, 0, :] = embW[1]
    words[:, 1:, :] = embW[caps[:, :-1]]

    xpad = np.zeros((H, TB), np.float32)                  # [1024, 400]
    xpad[:E] = words.transpose(2, 1, 0).reshape(E, TB)    # (e, t, b)
    xpad[E:XDIM] = np.broadcast_to(
        feats.T[:, None, :], (ENC, T, BS)).reshape(ENC, TB)
    xT_t = np.ascontiguousarray(
        xpad.reshape(KT, 128, TB).transpose(1, 0, 2).reshape(128, KT * TB)
    ).astype(p1np)
    return {"xT": xT_t}


_BUILT = {}


def kernel(features, captions, emb_W, W_ih, W_hh, b_ih, b_hh,
           enc_W, enc_b, dec_W, dec_b, full_W, full_b, fcn_W, fcn_b,
           _cfg=None, _trace=False):
    cfg = dict(CFG if _cfg is None else _cfg)
    key = (cfg["p1"], cfg["rec"], cfg["fcn"], cfg.get("heat", 1),
           cfg.get("ident", True))
    if key not in _BUILT:
        _BUILT[key] = build_nc(cfg)
    nc = _BUILT[key]

    shared = _prep_shared(W_ih, W_hh, b_ih, b_hh, fcn_W, fcn_b, cfg)
    in_maps = []
    for c in range(NCORES):
        m = dict(shared)
        m.update(_prep_core(features, captions, emb_W, c, cfg))
        in_maps.append(m)

    res = run_bass_kernel_spmd(nc, in_maps, list(range(NCORES)), trace=_trace)

    fb = np.asarray(fcn_b, np.float32)
    out = np.empty((B, T - 1, V), np.float32)
    for c in range(NCORES):
        o = np.asarray(res.results[c]["out"], dtype=np.float32)  # [NCH,128,CVT*ROWS]
        # cols = mi*ROWS + t*BS + b ; vocab = (chunk*CVT + mi)*128 + p
        o = o.reshape(NCH, 128, CVT, T - 1, BS)
        o = o.transpose(4, 3, 0, 2, 1).reshape(BS, T - 1, V)
        out[c * BS:(c + 1) * BS] = o
    out += fb[None, None, :]
    kernel._last_result = res
    return out
